# revision 26
# baseline (speedup 1.0000x reference)
"""Trainium2 Bass kernel for causal multi-head attention.

Problem: B=2, S=2048, D=2048, H=16 heads (HD=128), fp32, causal.
Sharding: 8 cores = 2 batches (data parallel) x 4 head-groups (tensor
parallel, 4 heads each). Each core computes Q/K/V projections for its
head slice, causal attention, and a partial out-projection; the host
sums the 4 partials per batch and adds the output bias.

Design notes (evolved from an fp32r baseline; HW-measured at each step):
  - All matmul operands are bf16: same PE rate as fp32r (1 cycle/row)
    but no N>=256 penalty on causal-diagonal tiles, half the DMA bytes,
    half the SBUF. Output written bf16 too (host upcasts + sums).
    Measured rel-err ~6e-3 against fp32 reference (tolerance 2e-2).
  - All weights are DMAed once and stay resident in SBUF; the DMA
    emission is priority-ordered (chunk-0 x interleaved with wq, then
    wk/wv/wo) so the first matmul starts ~1us in.
  - Softmax denominators (default 'dve16'): running ptsum accumulated
    in bf16 on the DVE (2x 16-bit rate, and the bf16 ptsum feeds the
    ones-matmul directly, no staging copy). HW A/B: dve16 444us <
    dve(f32+copy) 456 < pe(per-k-tile PE matmuls) 490 < gpsimd 507 --
    real HW charges a flat ~280ns per matmul instruction (measured;
    ~67ns fixed issue/LDW tax over the 213ns of N=512 streaming), so
    den='pe' loses by its +144 instructions, and the real GpSimd is
    slower than the Pool cost model claims.
  - fp8 DoubleRow measured at the SAME ~280ns per instruction (2x MACs
    via K=256): a hi/lo-split fp8 projection would only break even vs
    bf16, and single-rounded fp8 fails the error budget. Not used.
  - Emission is software-pipelined: sc(kt+1) issues before AV(kt) so
    the PE never waits on the scalar-engine exp; the normalization of
    head h issues interleaved with the first matmuls of head h+1.
  - cfg 'interleave' (proj chunk n then attention qt=n, per-segment
    PSUM pools) measured ~60us WORSE on HW; default off.
"""

import sys

if "/opt/trn_rl_repo" not in sys.path:
    sys.path.insert(0, "/opt/trn_rl_repo")

import ml_dtypes
import numpy as np

import concourse.bacc as bacc
import concourse.mybir as mybir
import concourse.tile as tile
from concourse.bass_utils import run_bass_kernel_spmd
from concourse.masks import make_upper_triangular

B, S, D, H = 2, 2048, 2048, 16
HD = 128                 # head dim
NCORES = 8
HPC = 4                  # heads per core
DC = HPC * HD            # 512: per-core projection width
CT = D // 128            # 16 contraction tiles
QT = S // 512            # 4 query chunks of 512
ST = S // 128            # 16 seq tiles of 128
SCALE = 1.0 / float(np.sqrt(HD))
F32 = mybir.dt.float32
BF16 = mybir.dt.bfloat16
EXP = mybir.ActivationFunctionType.Exp
NPBF16 = ml_dtypes.bfloat16

DEFAULT_CFG = {"den": "dve16"}

_BUILT = None


def _build(cfg=None, reps=1):
    cfg = dict(DEFAULT_CFG, **(cfg or {}))
    XCB = cfg.get("xcb", 2)    # x-chunk double buffering
    PTB = cfg.get("ptb", 4)    # p^T tile bufs
    SCB = cfg.get("scb", 2)    # scores psum bufs
    CPB = cfg.get("cpb", 2)    # ctx psum bufs
    PPB = cfg.get("ppb", 2)    # proj psum bufs
    DEN = cfg.get("den", "gpsimd")  # denom: 'gpsimd'|'dve'|'dve16'|'pe'
    OTE = cfg.get("ot", "dve")      # out-tile copy engine: 'dve'|'scalar'
    INTER = cfg.get("interleave", False)
    nc = bacc.Bacc(trn_type="TRN2", target_bir_lowering=False)
    MASK_ENG = (nc.gpsimd if (DEN == "gpsimd" or cfg.get("mask") == "gpsimd")
                else nc.vector)
    SUM_ENG = nc.gpsimd if DEN == "gpsimd" else nc.vector
    PTS_DT = BF16 if DEN == "dve16" else F32
    xT_d = nc.dram_tensor("xT", [D, S], BF16, kind="ExternalInput")
    wqT_d = nc.dram_tensor("wqT", [D, DC], BF16, kind="ExternalInput")
    wkT_d = nc.dram_tensor("wkT", [D, DC], BF16, kind="ExternalInput")
    wvT_d = nc.dram_tensor("wvT", [D, DC], BF16, kind="ExternalInput")
    woT_d = nc.dram_tensor("woT", [DC, D], BF16, kind="ExternalInput")
    out_d = nc.dram_tensor("out", [S, D], BF16, kind="ExternalOutput")

    with tile.TileContext(nc) as tc:
      for _rep in range(reps):
        _p = f"r{_rep}_"
        with (
            tc.tile_pool(name=_p + "const", bufs=1) as cst,
            tc.tile_pool(name=_p + "persist", bufs=1) as pp,
            tc.tile_pool(name=_p + "weights", bufs=1) as wp,
            tc.tile_pool(name=_p + "xc", bufs=XCB) as xcp,
            tc.tile_pool(name=_p + "ptp", bufs=PTB) as ptp,
            tc.tile_pool(name=_p + "pts", bufs=2) as pts,
            tc.tile_pool(name=_p + "ptb", bufs=2) as ptbp,
            tc.tile_pool(name=_p + "rcp", bufs=2) as rcp,
            tc.tile_pool(name=_p + "rbs", bufs=2) as rbsp,
            tc.tile_pool(name=_p + "osb", bufs=3) as osp,
        ):
            # upper-triangular (incl diagonal) 0/1 mask: allowed = k <= q
            tri_f = cst.tile([128, 128], F32, tag="tri_f", name="tri_f")
            make_upper_triangular(nc, tri_f[:], val=1.0, diag=True)
            tri = cst.tile([128, 128], BF16, tag="tri", name="tri")
            nc.vector.tensor_copy(tri[:], tri_f[:])
            ones_f = cst.tile([128, 1], F32, tag="ones_f", name="ones_f")
            nc.vector.memset(ones_f[:], 1.0)
            ones_col = cst.tile([128, 1], BF16, tag="ones_col", name="ones_col")
            nc.vector.tensor_copy(ones_col[:], ones_f[:])
            ones_rf = cst.tile([1, 128], F32, tag="ones_rf", name="ones_rf")
            nc.vector.memset(ones_rf[:], 1.0)
            ones_row = cst.tile([1, 128], BF16, tag="ones_row", name="ones_row")
            nc.vector.tensor_copy(ones_row[:], ones_rf[:])

            # persistent per-core tensors (partition dim x free dim):
            # qT/kT: per head [HD, S]; v: per s-tile [128, DC]; ctx^T per
            # (head, q-chunk) for fine-grained deps.
            qTt = [pp.tile([128, S], BF16, tag=f"qT{h}", name=f"qT{h}") for h in range(HPC)]
            kTt = [pp.tile([128, S], BF16, tag=f"kT{h}", name=f"kT{h}") for h in range(HPC)]
            vt = [pp.tile([128, DC], BF16, tag=f"v{s}", name=f"v{s}") for s in range(ST)]
            ctxt = [[pp.tile([128, 512], BF16, tag=f"ctx{h}_{q}", name=f"ctx{h}_{q}")
                     for q in range(QT)] for h in range(HPC)]

            # Priority-ordered resident-weight + chunk-0 x DMA: interleave
            # chunk-0 x tiles with wq so the first Q-group matmuls start
            # ~1us in; wk/wv/wo stream behind and arrive before their
            # first consumers.
            wq_t, wk_t, wv_t = [], [], []
            wots = {}
            xcs0 = []
            for ct in range(CT):
                xc = xcp.tile([128, 512], BF16, tag=f"xc{ct}", name=f"xc_0_{ct}")
                nc.sync.dma_start(out=xc[:], in_=xT_d[ct * 128:(ct + 1) * 128, 0:512])
                xcs0.append(xc)
                w_sb = wp.tile([128, DC], BF16, tag=f"wq{ct}", name=f"wq{ct}")
                nc.sync.dma_start(out=w_sb[:], in_=wqT_d[ct * 128:(ct + 1) * 128, :])
                wq_t.append(w_sb)
            for w_d, lst, nm in ((wkT_d, wk_t, "wk"), (wvT_d, wv_t, "wv")):
                for ct in range(CT):
                    w_sb = wp.tile([128, DC], BF16, tag=f"{nm}{ct}", name=f"{nm}{ct}")
                    nc.sync.dma_start(out=w_sb[:], in_=w_d[ct * 128:(ct + 1) * 128, :])
                    lst.append(w_sb)
            for i in range(HPC):
                wo_sb = wp.tile([128, D], BF16, tag=f"wo{i}", name=f"wo_{i}")
                nc.sync.dma_start(out=wo_sb[:], in_=woT_d[i * 128:(i + 1) * 128, :])
                for oc in range(4):
                    wots[(oc, i)] = wo_sb[:, oc * 512:(oc + 1) * 512]

            # ---------------- projection chunk emitter ----------------
            def emit_proj(n, pps):
                if n == 0:
                    xcs = xcs0
                else:
                    xcs = []
                    for ct in range(CT):
                        xc = xcp.tile([128, 512], BF16, tag=f"xc{ct}", name=f"xc_{n}_{ct}")
                        nc.sync.dma_start(
                            out=xc[:],
                            in_=xT_d[ct * 128:(ct + 1) * 128, n * 512:(n + 1) * 512],
                        )
                        xcs.append(xc)

                # Q^T and K^T: out[d-tile(=head) 128, s 512] accum over ct
                for w_tiles, dst in ((wq_t, qTt), (wk_t, kTt)):
                    acc = [pps.tile([128, 512], F32, tag=f"acc{m}", name=f"acc_{n}_{m}")
                           for m in range(HPC)]
                    for ct in range(CT):
                        for m in range(HPC):
                            nc.tensor.matmul(
                                acc[m][:],
                                (w_tiles[ct][:, m * 128:(m + 1) * 128]),
                                (xcs[ct][:]),
                                start=(ct == 0),
                                stop=(ct == CT - 1),
                            )
                    for m in range(HPC):
                        nc.vector.tensor_copy(
                            dst[m][:, n * 512:(n + 1) * 512], acc[m][:]
                        )

                # V natural [s-tile 128, d 512]: lhsT = x^T chunk, rhs = wv^T
                accv = [pps.tile([128, 512], F32, tag=f"acc{ss}", name=f"accv_{n}_{ss}")
                        for ss in range(4)]
                for ct in range(CT):
                    for ss in range(4):
                        nc.tensor.matmul(
                            accv[ss][:],
                            (xcs[ct][:, ss * 128:(ss + 1) * 128]),
                            (wv_t[ct][:]),
                            start=(ct == 0),
                            stop=(ct == CT - 1),
                        )
                for ss in range(4):
                    nc.vector.tensor_copy(vt[n * 4 + ss][:], accv[ss][:])

            # ---------------- attention q-chunk emitter ----------------
            def emit_attn(qt, scp, cxp, dnp, rbp, ops):
                nkt = 4 * qt + 4  # causal: k-tiles 0..4qt+3

                def norm_den(st):
                    # one ones-matmul over the accumulated ptsum
                    # (no-op for DEN='pe': den accumulated per k-tile)
                    if DEN == "pe":
                        return
                    h_ = st["h"]
                    den = dnp.tile([1, 512], F32, tag="den", name=f"den_{h_}_{qt}")
                    nc.tensor.matmul(den[:], (ones_col[:]), (st["ptb"][:]),
                                     start=True, stop=True)
                    st["den"] = den

                def norm_dve(st):
                    h_ = st["h"]
                    recip = rcp.tile([1, 512], BF16, tag="recip",
                                     name=f"recip_{h_}_{qt}")
                    with nc.allow_low_precision("softmax denom recip in bf16"):
                        nc.vector.reciprocal(recip[:], st["den"][:])
                    st["recip"] = recip

                def norm_rb(st):
                    h_ = st["h"]
                    rb = rbp.tile([128, 512], F32, tag="rb", name=f"rb_{h_}_{qt}")
                    nc.tensor.matmul(rb[:], (ones_row[:]), (st["recip"][:]),
                                     start=True, stop=True)
                    # HW allows only one PSUM operand per vector op, and
                    # GPSIMD can't read PSUM: stage rb via DVE.
                    rbsb = rbsp.tile([128, 512], F32, tag="rbs",
                                     name=f"rbs_{h_}_{qt}")
                    nc.vector.tensor_copy(rbsb[:], rb[:])
                    ctx = ctxt[h_][qt]
                    nc.vector.tensor_mul(ctx[:], st["cps"][:], rbsb[:])

                pending = None
                for h in range(HPC):
                    cps = cxp.tile([128, 512], F32, tag="cps", name=f"cps_{h}_{qt}")
                    if DEN == "pe":
                        den_acc = dnp.tile([1, 512], F32, tag="den",
                                           name=f"den_{h}_{qt}")
                        ptsum = None
                    else:
                        den_acc = None
                        ptsum = pts.tile([128, 512], PTS_DT, tag="ptsum",
                                         name=f"ptsum_{h}_{qt}")
                    pt_tiles = [None] * nkt

                    def emit_sc(kt):
                        j = kt - 4 * qt
                        lo = 0 if j < 0 else j * 128
                        sc = scp.tile([128, 512], F32, tag="sc",
                                      name=f"sc_{h}_{qt}_{kt}")
                        nc.tensor.matmul(
                            sc[:, lo:],
                            (kTt[h][:, kt * 128:(kt + 1) * 128]),
                            (qTt[h][:, qt * 512 + lo:(qt + 1) * 512]),
                            start=True,
                            stop=True,
                        )
                        pt = ptp.tile([128, 512], BF16, tag="pt",
                                      name=f"pt_{h}_{qt}_{kt}")
                        nc.scalar.activation(pt[:, lo:], sc[:, lo:], EXP,
                                             scale=SCALE)
                        if j >= 0:
                            # strictly-diagonal 128x128 sub-block mask
                            MASK_ENG.tensor_mul(
                                pt[:, j * 128:(j + 1) * 128],
                                pt[:, j * 128:(j + 1) * 128],
                                tri[:],
                            )
                        if DEN != "pe":
                            if kt == 0:
                                SUM_ENG.tensor_copy(ptsum[:], pt[:])
                            else:
                                SUM_ENG.tensor_add(ptsum[:, lo:], ptsum[:, lo:],
                                                   pt[:, lo:])
                        pt_tiles[kt] = pt

                    def emit_av(kt):
                        j = kt - 4 * qt
                        lo = 0 if j < 0 else j * 128
                        if DEN == "pe":
                            nc.tensor.matmul(
                                den_acc[:, lo:], (ones_col[:]),
                                (pt_tiles[kt][:, lo:]),
                                start=(kt == 0), stop=(kt == nkt - 1),
                            )
                        nc.tensor.matmul(
                            cps[:, lo:],
                            (vt[kt][:, h * 128:(h + 1) * 128]),
                            (pt_tiles[kt][:, lo:]),
                            start=(kt == 0),
                            stop=(kt == nkt - 1),
                        )

                    # pipelined emission: sc runs one k-tile ahead of AV;
                    # prev head's den/recip/rb slot between them so the
                    # PE never waits on the scalar/vector engines.
                    emit_sc(0)
                    emit_sc(1)
                    if pending is not None:
                        norm_den(pending)
                        if DEN == "pe":
                            # recip(h-1) must be emitted before emit_av(0)
                            # writes den_acc(h): the single den PSUM buffer
                            # is reused, and the reuse dependency is only
                            # tracked in emission order.
                            norm_dve(pending)
                    emit_av(0)
                    if pending is not None and DEN != "pe":
                        norm_dve(pending)
                    emit_sc(2)
                    if pending is not None:
                        norm_rb(pending)
                    emit_av(1)
                    for kt in range(2, nkt - 1):
                        emit_sc(kt + 1)
                        emit_av(kt)
                    emit_av(nkt - 1)
                    pending = {"h": h, "cps": cps}
                    if DEN == "pe":
                        pending["den"] = den_acc
                    elif DEN == "dve16":
                        # bf16 ptsum is a legal matmul operand directly
                        pending["ptb"] = ptsum
                    else:
                        ptsum_b = ptbp.tile([128, 512], BF16, tag="ptb",
                                            name=f"ptb_{h}_{qt}")
                        SUM_ENG.tensor_copy(ptsum_b[:], ptsum[:])
                        pending["ptb"] = ptsum_b

                # flush last head's normalization before the out-projection
                norm_den(pending)
                norm_dve(pending)
                norm_rb(pending)

                # out-projection for this query chunk (4 q-tiles of 128).
                # bf16 output tiles paired into [128,1024] for 2KB DMA
                # lines and half the write traffic; host upcasts + sums.
                for r in range(4):
                    q = qt * 4 + r
                    for op_ in range(2):
                        ot = osp.tile([128, 1024], BF16, tag="ot", name=f"ot_{op_}_{q}")
                        for half in range(2):
                            oc = op_ * 2 + half
                            po = ops.tile([128, 512], F32, tag="po", name=f"po_{oc}_{q}")
                            for i in range(HPC):
                                nc.tensor.matmul(
                                    po[:],
                                    (ctxt[i][qt][:, r * 128:(r + 1) * 128]),
                                    (wots[(oc, i)]),
                                    start=(i == 0),
                                    stop=(i == HPC - 1),
                                )
                            if OTE == "scalar":
                                nc.scalar.copy(
                                    ot[:, half * 512:(half + 1) * 512], po[:]
                                )
                            else:
                                nc.vector.tensor_copy(
                                    ot[:, half * 512:(half + 1) * 512], po[:]
                                )
                        nc.sync.dma_start(
                            out=out_d[q * 128:(q + 1) * 128,
                                      op_ * 1024:(op_ + 1) * 1024],
                            in_=ot[:],
                        )

            def attn_pools(tag):
                return (
                    tc.tile_pool(name=_p + f"sc_ps{tag}", bufs=SCB, space="PSUM"),
                    tc.tile_pool(name=_p + f"ctx_ps{tag}", bufs=CPB, space="PSUM"),
                    tc.tile_pool(name=_p + f"den_ps{tag}", bufs=1, space="PSUM"),
                    tc.tile_pool(name=_p + f"rb_ps{tag}", bufs=1, space="PSUM"),
                    tc.tile_pool(name=_p + f"out_ps{tag}", bufs=2, space="PSUM"),
                )

            if INTER:
                from contextlib import ExitStack
                for seg in range(QT):
                    with tc.tile_pool(name=_p + f"pj_ps{seg}", bufs=PPB,
                                      space="PSUM") as pps:
                        emit_proj(seg, pps)
                    with ExitStack() as es:
                        pools = [es.enter_context(p) for p in attn_pools(seg)]
                        emit_attn(seg, *pools)
            else:
                with tc.tile_pool(name=_p + "pj_ps", bufs=PPB, space="PSUM") as pps:
                    for n in range(QT):
                        emit_proj(n, pps)
                from contextlib import ExitStack
                with ExitStack() as es:
                    pools = [es.enter_context(p) for p in attn_pools("")]
                    for qt in range(QT):
                        emit_attn(qt, *pools)

    nc.compile()
    return nc


def _get_built():
    global _BUILT
    if _BUILT is None:
        _BUILT = _build()
    return _BUILT


def make_in_maps(x, wq, wk, wv, wo):
    x = np.asarray(x, dtype=np.float32)
    wq = np.asarray(wq, dtype=np.float32)
    wk = np.asarray(wk, dtype=np.float32)
    wv = np.asarray(wv, dtype=np.float32)
    wo = np.asarray(wo, dtype=np.float32)
    in_maps = []
    for c in range(NCORES):
        b, hg = divmod(c, NCORES // B)
        sl = slice(hg * DC, (hg + 1) * DC)
        in_maps.append({
            "xT": np.ascontiguousarray(x[b].T).astype(NPBF16),
            "wqT": np.ascontiguousarray(wq[sl, :].T).astype(NPBF16),
            "wkT": np.ascontiguousarray(wk[sl, :].T).astype(NPBF16),
            "wvT": np.ascontiguousarray(wv[sl, :].T).astype(NPBF16),
            "woT": np.ascontiguousarray(wo[:, sl].T).astype(NPBF16),
        })
    return in_maps


def combine_outputs(results, bo):
    bo = np.asarray(bo, dtype=np.float32)
    out = np.zeros((B, S, D), dtype=np.float32)
    for c in range(NCORES):
        b = c // (NCORES // B)
        out[b] += np.asarray(results[c]["out"], dtype=np.float32)
    out += bo[None, None, :]
    return out


def kernel(x, wq, wk, wv, wo, bo):
    nc = _get_built()
    in_maps = make_in_maps(x, wq, wk, wv, wo)
    res = run_bass_kernel_spmd(nc, in_maps, core_ids=list(range(NCORES)))
    return combine_outputs(res.results, bo)


if __name__ == "__main__":
    nc = _get_built()
    print("built ok; instructions:", len(nc.inst_map))
